# revision 3
# baseline (speedup 1.0000x reference)
"""EnhancedGATCN Trainium2 kernel: 2-layer GAT (heads=1, edge attrs) + linear head.

Strategy (8 NeuronCores, SPMD), v2:
  - Destination-node sharding: core k owns dst nodes [k*12544, (k+1)*12544).
  - Per layer a per-node table [h(64)|1|asrc|adst] is built on-chip, stored
    as fp16 rows padded to 128 elems (256B) and AllGathered; each core then
    dma_gathers, per edge, the src row from the full table and the dst row
    from its local shard, computes edge softmax numerators
    ex = exp(leaky_relu(asrc+adst+ae)), and aggregates per dst tile with a
    one-hot scatter matmul psum[128 dst, 65] += onehot^T @ (ex*[h|1]).
    Segment max is skipped (logits O(10); exp safe in f32, max cancels).
  - W2 applied after aggregation (linearity); final linear fused into the
    layer-2 drain in transposed layout.

Perf notes (v1 -> v2, ~89ms -> ~74ms printed incl. the ~72-75ms axon
dispatch floor; true device time ~17ms -> ~6ms):
  - fp16 256B table rows halve gather bytes vs f32 512B (gather rate is
    byte-bound, not latency-bound: ~4ns/row fp16 vs ~8ns f32).
  - dma_gather calls spread over all 4 SWDGE queues (num_swdge_queues=4,
    queue_num round-robin) - the single-queue serialization was the real
    v1 bottleneck (~3.3x on the edge pass); each gather call further split
    in two (split=2) for finer queue interleave.
  - one-hot scatter matrices in bf16 and ex*[h|1] in fp16 (2x DVE rate,
    16-bit PE matmul); psum accumulation stays f32. Mixed-dtype DVE ops
    consume the fp16 gathered payload directly in the f32 z/exp pipeline.
  - measured: Relative L2 error ~8e-5 vs the f32 jax reference (fp16
    table quantization dominates; gate is 2e-2).

Host-side prep (numpy): shard + sort edges by (dst tile pair, src segment),
pad each (tile,seg) run to a fixed RUN so all 8 cores share one SPMD
instruction stream; int16 gather indices (4 src segments of 25088 rows so
indices fit int16) and f32/bf16 slot arrays.
"""

import numpy as np

N = 100_000
E = 3_200_000
IN_CH, HID, EXT = 128, 64, 3
NEG_SLOPE = 0.2
CORES = 8
NC_NODES = 12544            # 98 tiles of 128 per core
NTILES = NC_NODES // 128    # 98
NPAD = CORES * NC_NODES     # 100352
SEGS = 4
SEG_ROWS = NPAD // SEGS     # 25088  (< 32768 so int16 indices fit)
NPAIR = NTILES // 2         # 49 tile pairs per core
TBL_W = 128                 # fp16 table row width -> 256B rows


def _prep(x, x_ext, edge_index, edge_weight,
          W1, att_src1, att_dst1, We1, att_e1, b1,
          W2, att_src2, att_dst2, We2, att_e2, b2,
          Wlin, blin, want_dstidx=False):
    """Host prep. Returns (per_core_inputs: list[dict], consts: dict, run)."""
    x = np.asarray(x, np.float32)
    x_ext = np.asarray(x_ext, np.float32)
    src = np.asarray(edge_index[0], np.int64)
    dst = np.asarray(edge_index[1], np.int64)
    w = np.asarray(edge_weight, np.float32).reshape(-1)

    k1 = float(np.asarray(We1, np.float32).reshape(-1) @ np.asarray(att_e1, np.float32))
    k2 = float(np.asarray(We2, np.float32).reshape(-1) @ np.asarray(att_e2, np.float32))

    core = dst // NC_NODES
    tile_l = (dst % NC_NODES) // 128        # 0..97
    seg = src // SEG_ROWS                   # 0..3
    # group index within core: (tile//2)*8 + seg*2 + (tile&1)
    grp = (tile_l // 2) * 8 + seg * 2 + (tile_l & 1)
    NGRP = NPAIR * 8  # 392

    flat = core * NGRP + grp
    counts = np.bincount(flat, minlength=CORES * NGRP)
    run = int(np.ceil(counts.max() / 128.0) * 128)
    S = NGRP * run

    xcat = np.concatenate([x, x_ext], axis=1)          # [N, 131]
    xcat_pad = np.zeros((NPAD, IN_CH + EXT), np.float32)
    xcat_pad[:N] = xcat

    order = np.argsort(flat, kind="stable")
    fs = flat[order]
    cum = np.zeros(CORES * NGRP + 1, np.int64)
    np.cumsum(counts, out=cum[1:])
    rank = np.arange(E, dtype=np.int64) - cum[fs]
    slot = (fs % NGRP) * run + rank                    # slot within its core
    core_o = fs // NGRP

    per_core = []
    for k in range(CORES):
        m = core_o == k
        sl = slot[m]
        e = order[m]
        src16 = np.zeros(S, np.int16)
        dst16 = np.zeros(S, np.int16)
        dstrow = np.full(S, -1.0, np.float32)
        ae1 = np.zeros(S, np.float32)
        ae2 = np.zeros(S, np.float32)
        src16[sl] = (src[e] - seg[e] * SEG_ROWS).astype(np.int16)
        dst16[sl] = (dst[e] % NC_NODES).astype(np.int16)
        dstrow[sl] = (dst[e] % 128).astype(np.float32)
        ae1[sl] = k1 * w[e]
        ae2[sl] = k2 * w[e]

        def wrap16(a):  # [128, S/16] replicated for the 8 Q7 cores
            return np.tile(a.reshape(S // 16, 16).T, (8, 1)).copy()

        def wrap128(a):  # [128, S/128], slot s -> [s%128, s//128]
            return a.reshape(S // 128, 128).T.copy()

        xcT = xcat_pad[k * NC_NODES:(k + 1) * NC_NODES].T.copy()  # [131, NC]
        d = {
            "srcidx": wrap16(src16),
            "dstrow": wrap128(dstrow),
            "ae1": wrap128(ae1),
            "ae2": wrap128(ae2),
            "xa": np.ascontiguousarray(xcT[:IN_CH]),       # [128, NC]
            "xb": np.ascontiguousarray(xcT[IN_CH:]),       # [3, NC]
        }
        if want_dstidx:
            d["dstidx"] = wrap16(dst16)
        per_core.append(d)

    W1 = np.asarray(W1, np.float32)
    W2 = np.asarray(W2, np.float32)
    Wlin = np.asarray(Wlin, np.float32)
    consts = {
        "w1a": np.ascontiguousarray(W1[:IN_CH]),           # [128, 64]
        "w1b": np.ascontiguousarray(W1[IN_CH:]),           # [3, 64]
        "asd1": np.stack([np.asarray(att_src1, np.float32),
                          np.asarray(att_dst1, np.float32)], 1),  # [64, 2]
        "w2": W2,                                          # [64, 64] (lhsT)
        "avs2": np.tile(W2 @ np.asarray(att_src2, np.float32), (128, 1)),  # [128,64]
        "avd2": np.tile(W2 @ np.asarray(att_dst2, np.float32), (128, 1)),  # [128,64]
        "b1rep": np.tile(np.asarray(b1, np.float32), (128, 1)),   # [128, 64]
        "b2col": np.asarray(b2, np.float32).reshape(HID, 1),      # [64, 1]
        "wlina": np.ascontiguousarray(Wlin[:HID]),         # [64, 2]
        "wlinb": np.ascontiguousarray(Wlin[HID:]),         # [3, 2]
        "blincol": np.asarray(blin, np.float32).reshape(2, 1),    # [2, 1]
        "iota": np.tile(np.arange(128, dtype=np.float32), (128, 1)),  # [128,128]
        "ident": np.eye(128, dtype=np.float32),
    }
    return per_core, consts, run


# ============================ BASS PROGRAM ============================

_PROG_CACHE = {}


def _build_program(run, use_dst_gather=False, stage="full", rep=1,
                   n_devices=CORES, nq=1, egbufs=2, split=1, lite16=False):
    import concourse.bacc as bacc
    import concourse.mybir as mybir
    import concourse.tile as tile
    dt = mybir.dt
    f32 = dt.float32
    f16 = dt.float16
    bf16 = dt.bfloat16
    onedt = bf16 if lite16 else f32
    mexdt = f16 if lite16 else f32

    RUN_CH = run // 128
    C = 8 * RUN_CH            # chunks per pair-group
    GRP = 8 * run             # slots per pair-group
    S = NPAIR * GRP

    nc = bacc.Bacc("TRN2", target_bir_lowering=False, debug=False,
                   num_devices=n_devices, num_swdge_queues=nq)

    def din(name, shape, d=f32):
        return nc.dram_tensor(name, shape, d, kind="ExternalInput")

    srcidx_d = din("srcidx", [128, S // 16], dt.int16)
    dstidx_d = din("dstidx", [128, S // 16], dt.int16) if use_dst_gather else None
    dstrow_d = din("dstrow", [128, S // 128])
    ae_d = [din("ae1", [128, S // 128]), din("ae2", [128, S // 128])]
    xa_d = din("xa", [128, NC_NODES])
    xb_d = din("xb", [3, NC_NODES])
    w1a_d = din("w1a", [IN_CH, HID])
    w1b_d = din("w1b", [3, HID])
    asd1_d = din("asd1", [HID, 2])
    w2_d = din("w2", [HID, HID])
    avs2_d = din("avs2", [128, HID])
    avd2_d = din("avd2", [128, HID])
    b1rep_d = din("b1rep", [128, HID])
    b2col_d = din("b2col", [HID, 1])
    wlina_d = din("wlina", [HID, 2])
    wlinb_d = din("wlinb", [3, 2])
    blin_d = din("blincol", [2, 1])
    iota_d = din("iota", [128, 128])
    ident_d = din("ident", [128, 128])
    out_d = nc.dram_tensor("out", [2, NC_NODES], f32, kind="ExternalOutput")
    dbg_d = None
    if stage in ("phasea", "ag", "l1dump"):
        dbg_d = nc.dram_tensor("dbg", [NC_NODES, TBL_W], f32,
                               kind="ExternalOutput")

    AX = mybir.AxisListType
    OP = mybir.AluOpType
    AF = mybir.ActivationFunctionType

    with tile.TileContext(nc) as tc:
        with (
            tc.tile_pool(name="dram", bufs=1, space="DRAM") as dram,
            tc.tile_pool(name="const", bufs=1) as cpool,
            tc.tile_pool(name="persist", bufs=1) as ppool,
        ):
            town = [dram.tile([NC_NODES, TBL_W], f16, name=f"town{i}")
                    for i in range(2)]
            tfull = [dram.tile([NPAD, TBL_W], f16, name=f"tfull{i}",
                              addr_space="Shared")
                     for i in range(2)]

            iota_sb = cpool.tile([128, 128], f32)
            ident_sb = cpool.tile([128, 128], f32)
            w2_sb = cpool.tile([HID, HID], f32)
            avs2_sb = cpool.tile([128, HID], f32)
            avd2_sb = cpool.tile([128, HID], f32)
            b1rep_sb = cpool.tile([128, HID], f32)
            b2col_sb = cpool.tile([HID, 1], f32)
            wlina_sb = cpool.tile([HID, 2], f32)
            wlinb_sb = cpool.tile([3, 2], f32)
            blin_sb = cpool.tile([2, 1], f32)
            xb_sb = ppool.tile([3, NC_NODES], f32)
            iota_b = cpool.tile([128, 128], bf16)
            # resident per-tile adst tables for this core's own dst nodes
            adT = [ppool.tile([128, NTILES], f32, name=f"adT{i}")
                   for i in range(2)]
            for sb, d in [(iota_sb, iota_d), (ident_sb, ident_d),
                          (w2_sb, w2_d), (avs2_sb, avs2_d), (avd2_sb, avd2_d),
                          (b1rep_sb, b1rep_d), (b2col_sb, b2col_d),
                          (wlina_sb, wlina_d), (wlinb_sb, wlinb_d),
                          (blin_sb, blin_d), (xb_sb, xb_d)]:
                nc.sync.dma_start(sb[:], d[:])
            nc.vector.tensor_copy(iota_b[:], iota_sb[:])

            # ---------------- phase A: layer-1 table ----------------
            with (
                tc.tile_pool(name="pa", bufs=2) as pa,
                tc.tile_pool(name="pa_ps", bufs=2, space="PSUM") as pa_ps,
                tc.tile_pool(name="pa1", bufs=1) as pa1,
            ):
                xa_sb = pa1.tile([128, NC_NODES], f32)
                nc.sync.dma_start(xa_sb[:], xa_d[:])
                w1a_sb = pa1.tile([IN_CH, HID], f32)
                w1b_sb = pa1.tile([3, HID], f32)
                asd1_sb = pa1.tile([HID, 2], f32)
                nc.sync.dma_start(w1a_sb[:], w1a_d[:])
                nc.sync.dma_start(w1b_sb[:], w1b_d[:])
                nc.sync.dma_start(asd1_sb[:], asd1_d[:])

                hT = pa1.tile([HID, NC_NODES], f32)
                asd_own = pa1.tile([2, NC_NODES], f32)
                CK = 448
                for c in range(NC_NODES // CK):
                    sl = slice(c * CK, (c + 1) * CK)
                    ph = pa_ps.tile([HID, CK], f32, tag="ph")
                    nc.tensor.matmul(ph[:], w1a_sb[:], xa_sb[:, sl],
                                     start=True, stop=False)
                    nc.tensor.matmul(ph[:], w1b_sb[:], xb_sb[:, sl],
                                     start=False, stop=True)
                    nc.vector.tensor_copy(hT[:, sl], ph[:])
                    pa2 = pa_ps.tile([2, CK], f32, tag="pa2")
                    nc.tensor.matmul(pa2[:], asd1_sb[:], hT[:, sl],
                                     start=True, stop=True)
                    nc.vector.tensor_copy(asd_own[:, sl], pa2[:])

                for t in range(NTILES):
                    sl = slice(t * 128, (t + 1) * 128)
                    stg = pa.tile([128, 67], f16, tag="stg")
                    pt = pa_ps.tile([128, HID], f32, tag="pt")
                    nc.tensor.transpose(pt[:], hT[:, sl], ident_sb[:HID, :HID])
                    nc.vector.tensor_copy(stg[:, 0:HID], pt[:])
                    nc.vector.memset(stg[:, HID:HID + 1], 1.0)
                    pt2 = pa_ps.tile([128, 2], f32, tag="pt2")
                    nc.tensor.transpose(pt2[:], asd_own[:, sl], ident_sb[:2, :2])
                    nc.vector.tensor_copy(stg[:, HID + 1:HID + 3], pt2[:])
                    nc.vector.tensor_copy(adT[0][:, t:t + 1], pt2[:, 1:2])
                    nc.sync.dma_start(town[0][sl, 0:67], stg[:])

            # ---------------- per-layer edge pass ----------------
            def edge_layer(layer, ngroups=NPAIR, upto=9, ag=True,
                           drain=True, lsuf=""):
                if ag:
                    nc.gpsimd.collective_compute(
                        "AllGather", OP.bypass,
                        replica_groups=[list(range(CORES))],
                        ins=[town[layer].opt()],
                        outs=[tfull[layer].opt()],
                    )
                with (
                    tc.tile_pool(name=f"eg{layer}{lsuf}", bufs=egbufs) as eg,
                    tc.tile_pool(name=f"eb{layer}{lsuf}", bufs=3) as eb,
                    tc.tile_pool(name=f"eps{layer}{lsuf}", bufs=1,
                                 space="PSUM") as eps,
                    tc.tile_pool(name=f"ed{layer}{lsuf}", bufs=2) as ed,
                ):
                    for g in range(ngroups):
                        gsl16 = slice(g * GRP // 16, (g + 1) * GRP // 16)
                        gsl128 = slice(g * C, (g + 1) * C)
                        isrc = eg.tile([128, GRP // 16], dt.int16, tag="isrc")
                        nc.sync.dma_start(isrc[:], srcidx_d[:, gsl16])
                        if use_dst_gather:
                            idst = eg.tile([128, GRP // 16], dt.int16,
                                           tag="idst")
                            nc.sync.dma_start(idst[:], dstidx_d[:, gsl16])
                        drow = eg.tile([128, C], f32, tag="drow")
                        aesb = eg.tile([128, C], f32, tag="aesb")
                        nc.sync.dma_start(drow[:], dstrow_d[:, gsl128])
                        nc.sync.dma_start(aesb[:], ae_d[layer][:, gsl128])

                        if upto < 1:
                            continue
                        gs = eg.tile([128, C * TBL_W], f16, tag="gs")
                        gs3 = gs[:].rearrange("p (c e) -> p c e", e=TBL_W)
                        nsp = 2 * run // split
                        for j in range(SEGS):
                            for h in range(split):
                                csl = slice(j * 2 * RUN_CH + h * nsp // 128,
                                            j * 2 * RUN_CH + (h + 1) * nsp // 128)
                                i0 = j * 2 * run // 16 + h * nsp // 16
                                nc.gpsimd.dma_gather(
                                    gs3[:, csl, :],
                                    tfull[layer][j * SEG_ROWS:(j + 1) * SEG_ROWS, :],
                                    isrc[:, i0:i0 + nsp // 16],
                                    nsp, nsp, TBL_W, elem_step=TBL_W,
                                    single_packet=False,
                                    queue_num=(j * split + h) % nq)
                        if use_dst_gather:
                            gd = eg.tile([128, C * TBL_W], f16, tag="gd")
                            gd3 = gd[:].rearrange("p (c e) -> p c e", e=TBL_W)
                            for j in range(SEGS):
                                for h in range(split):
                                    csl = slice(j * 2 * RUN_CH + h * nsp // 128,
                                                j * 2 * RUN_CH + (h + 1) * nsp // 128)
                                    i0 = j * 2 * run // 16 + h * nsp // 16
                                    nc.gpsimd.dma_gather(
                                        gd3[:, csl, :], town[layer][:, :],
                                        idst[:, i0:i0 + nsp // 16],
                                        nsp, nsp, TBL_W, elem_step=TBL_W,
                                        single_packet=False,
                                        queue_num=(j * split + h + 2) % nq)
                        if upto < 3:
                            continue
                        als = gs3[:, :, HID + 1:HID + 2].rearrange(
                            "p c e -> p (c e)")
                        if use_dst_gather:
                            ad = gd3[:, :, HID + 2:HID + 3].rearrange(
                                "p c e -> p (c e)")
                            zs = eg.tile([128, C], f32, tag="zs")
                            exs = eg.tile([128, C], f32, tag="exs")
                            nc.vector.tensor_tensor(zs[:], als, ad, OP.add)
                            nc.vector.tensor_tensor(zs[:], zs[:], aesb[:],
                                                    OP.add)
                            nc.vector.tensor_scalar(exs[:], zs[:], NEG_SLOPE,
                                                    None, OP.mult)
                            nc.vector.tensor_tensor(exs[:], exs[:], zs[:],
                                                    OP.max)
                            nc.scalar.activation(exs[:], exs[:], AF.Exp)
                        if upto < 4:
                            continue
                        pts = [eps.tile([128, HID + 1], f32, tag="acc",
                                        bufs=4 if use_dst_gather else 2,
                                        name=f"acc{i}")
                               for i in range(2)]
                        if lite16:
                            drow_b = eg.tile([128, C], bf16, tag="drowb")
                            nc.vector.tensor_copy(drow_b[:], drow[:])
                            exs_h = eg.tile([128, C], f16, tag="exsh")
                            nc.vector.tensor_copy(exs_h[:], exs[:])
                        for r in range(8):
                            j, i = r // 2, r % 2
                            rsl = slice(r * RUN_CH, (r + 1) * RUN_CH)
                            bt = eb.tile([128, RUN_CH * 128], onedt, tag="bt",
                                         bufs=3)
                            bt3 = bt[:].rearrange("p (c e) -> p c e", e=128)
                            nc.vector.tensor_tensor(
                                bt3[:, :, :],
                                (iota_b if lite16 else iota_sb)[:]
                                .rearrange("p (q e) -> p q e", q=1)
                                .broadcast_to([128, RUN_CH, 128]),
                                (drow_b[:, rsl] if lite16 else drow[:, rsl])
                                .to_broadcast([128, RUN_CH, 128]),
                                OP.is_equal)
                            if not use_dst_gather:
                                # adst per edge = btT^T @ adT[:, tile]
                                t_i = 2 * g + i
                                ad_ps = eps.tile([128, RUN_CH], f32,
                                                 tag="adps", bufs=2)
                                for cc in range(RUN_CH):
                                    btT_ps = eps.tile([128, 128], f32,
                                                      tag="btTp", bufs=1)
                                    nc.tensor.transpose(
                                        btT_ps[:], bt3[:, cc, :], ident_sb[:])
                                    btT = ed.tile([128, 128], f32, tag="btT")
                                    nc.vector.tensor_copy(btT[:], btT_ps[:])
                                    nc.tensor.matmul(
                                        ad_ps[:, cc:cc + 1], btT[:],
                                        adT[layer][:, t_i:t_i + 1],
                                        start=True, stop=True)
                                zs = eg.tile([128, RUN_CH], f32, tag="zs",
                                             bufs=3)
                                exs_t = eg.tile([128, RUN_CH], f32, tag="exs",
                                                bufs=3)
                                alsr = gs3[:, rsl, HID + 1:HID + 2].rearrange(
                                    "p c e -> p (c e)")
                                nc.vector.tensor_tensor(zs[:], alsr, ad_ps[:],
                                                        OP.add)
                                nc.vector.tensor_tensor(zs[:], zs[:],
                                                        aesb[:, rsl], OP.add)
                                nc.vector.tensor_scalar(exs_t[:], zs[:],
                                                        NEG_SLOPE, None,
                                                        OP.mult)
                                nc.vector.tensor_tensor(exs_t[:], exs_t[:],
                                                        zs[:], OP.max)
                                nc.scalar.activation(exs_t[:], exs_t[:],
                                                     AF.Exp)
                                exsl = exs_t[:]
                            else:
                                exsl = exs[:, rsl]
                            mex = eb.tile([128, RUN_CH * (HID + 1)], mexdt,
                                          tag="mex")
                            mex3 = mex[:].rearrange("p (c e) -> p c e",
                                                    e=HID + 1)
                            nc.vector.tensor_tensor(
                                mex3[:, :, :], gs3[:, rsl, 0:HID + 1],
                                (exs_h[:, rsl] if lite16 else exsl)
                                .to_broadcast(
                                    [128, RUN_CH, HID + 1]),
                                OP.mult)
                            for cc in range(RUN_CH):
                                nc.tensor.matmul(
                                    pts[i][:], bt3[:, cc, :],
                                    mex3[:, cc, :],
                                    start=(j == 0 and cc == 0),
                                    stop=(j == 3 and cc == RUN_CH - 1))

                        if upto < 5 or not drain:
                            continue
                        for i in range(2):
                            t = 2 * g + i
                            sl = slice(t * 128, (t + 1) * 128)
                            dsb = ed.tile([128, 1], f32, tag="dsb")
                            nc.vector.tensor_scalar(
                                dsb[:], pts[i][:, HID:HID + 1], 1e-16, None,
                                OP.add)
                            if layer == 0:
                                stg = ed.tile([128, 67], f16, tag="stg2")
                                tb = ed.tile([128, HID], f32, tag="tb")
                                nc.vector.tensor_scalar(
                                    tb[:], b1rep_sb[:], dsb[:], None, OP.mult)
                                nc.vector.tensor_tensor(
                                    tb[:], tb[:], pts[i][:, 0:HID], OP.add)
                                nc.scalar.activation(tb[:], tb[:], AF.Relu)
                                inv = ed.tile([128, 1], f32, tag="inv")
                                nc.vector.reciprocal(inv[:], dsb[:])
                                h1 = ed.tile([128, HID], f32, tag="h1")
                                nc.vector.tensor_scalar(
                                    h1[:], tb[:], inv[:], None, OP.mult)
                                nc.vector.tensor_copy(stg[:, 0:HID], h1[:])
                                nc.vector.memset(stg[:, HID:HID + 1], 1.0)
                                tmp = ed.tile([128, HID], f32, tag="tmp")
                                a2 = ed.tile([128, 2], f32, tag="a2")
                                nc.vector.tensor_tensor(
                                    tmp[:], h1[:], avs2_sb[:], OP.mult)
                                nc.vector.reduce_sum(
                                    a2[:, 0:1], tmp[:], axis=AX.X)
                                nc.vector.tensor_tensor(
                                    tmp[:], h1[:], avd2_sb[:], OP.mult)
                                nc.vector.reduce_sum(
                                    a2[:, 1:2], tmp[:], axis=AX.X)
                                nc.vector.tensor_copy(
                                    stg[:, HID + 1:HID + 3], a2[:])
                                nc.vector.tensor_copy(
                                    adT[1][:, t:t + 1], a2[:, 1:2])
                                nc.sync.dma_start(town[1][sl, 0:67], stg[:])
                            else:
                                msb = ed.tile([128, HID], f32, tag="msb")
                                inv = ed.tile([128, 1], f32, tag="inv")
                                nc.vector.reciprocal(inv[:], dsb[:])
                                nc.vector.tensor_scalar(
                                    msb[:], pts[i][:, 0:HID], inv[:], None,
                                    OP.mult)
                                pmT = eps.tile([HID, 128], f32, tag="pmT",
                                               bufs=1)
                                nc.tensor.transpose(pmT[:], msb[:],
                                                    ident_sb[:])
                                mT = ed.tile([HID, 128], f32, tag="mT")
                                nc.vector.tensor_copy(mT[:], pmT[:])
                                ph2 = eps.tile([HID, 128], f32, tag="ph2",
                                               bufs=1)
                                nc.tensor.matmul(ph2[:], w2_sb[:], mT[:],
                                                 start=True, stop=True)
                                h2T = ed.tile([HID, 128], f32, tag="h2T")
                                nc.scalar.activation(h2T[:], ph2[:], AF.Relu,
                                                     bias=b2col_sb[:])
                                po = eps.tile([2, 128], f32, tag="po", bufs=1)
                                nc.tensor.matmul(po[:], wlina_sb[:], h2T[:],
                                                 start=True, stop=False)
                                nc.tensor.matmul(po[:], wlinb_sb[:],
                                                 xb_sb[:, sl],
                                                 start=False, stop=True)
                                oT = ed.tile([2, 128], f32, tag="oT")
                                nc.scalar.activation(oT[:], po[:], AF.Relu,
                                                     bias=blin_sb[:])
                                nc.sync.dma_start(out_d[:, sl], oT[:])

            if stage == "phasea":
                with tc.tile_pool(name="dbgp", bufs=2) as dbgp:
                    for t in range(NTILES):
                        sl = slice(t * 128, (t + 1) * 128)
                        dd = dbgp.tile([128, TBL_W], f32, tag="dd")
                        dd16 = dbgp.tile([128, TBL_W], f16, tag="dd16")
                        nc.sync.dma_start(dd16[:], town[0][sl, :])
                        nc.vector.tensor_copy(dd[:], dd16[:])
                        nc.sync.dma_start(dbg_d[sl, :], dd[:])
            elif stage == "l1dump":
                edge_layer(0)
                with tc.tile_pool(name="dbgp", bufs=2) as dbgp:
                    for t in range(NTILES):
                        sl = slice(t * 128, (t + 1) * 128)
                        dd = dbgp.tile([128, TBL_W], f32, tag="dd")
                        dd16 = dbgp.tile([128, TBL_W], f16, tag="dd16")
                        nc.sync.dma_start(dd16[:], town[1][sl, :])
                        nc.vector.tensor_copy(dd[:], dd16[:])
                        nc.sync.dma_start(dbg_d[sl, :], dd[:])
            elif stage == "pre":
                tfp = [dram.tile([NPAD, TBL_W], f16, name=f"tfp{rr}",
                                 addr_space="Shared") for rr in range(rep)]
                for rr in range(rep):
                    nc.gpsimd.collective_compute(
                        "AllGather", OP.bypass,
                        replica_groups=[list(range(CORES))],
                        ins=[town[0].opt()],
                        outs=[tfp[rr].opt()],
                    )
            elif stage == "rep":
                # repeat layer-0 edge pass (no drain) `rep` times for slope
                # timing; single AllGather.
                for rr in range(rep):
                    edge_layer(0, ag=(rr == 0), drain=False, lsuf=f"r{rr}")
            elif stage == "repup":
                for rr in range(rep):
                    edge_layer(0, ag=False, drain=False, lsuf=f"r{rr}",
                               upto=2)
            else:
                edge_layer(0)
                edge_layer(1)

    nc.compile()
    return nc


def _get_program(run, use_dst_gather=False, nq=1, egbufs=2, split=1,
                 lite16=False):
    key = (run, use_dst_gather, nq, egbufs, split, lite16)
    if key not in _PROG_CACHE:
        _PROG_CACHE[key] = _build_program(run, use_dst_gather, nq=nq,
                                          egbufs=egbufs, split=split,
                                          lite16=lite16)
    return _PROG_CACHE[key]


def kernel(**inputs):
    from concourse.bass_utils import run_bass_kernel_spmd

    use_dst_gather = True
    per_core, consts, run = _prep(**inputs, want_dstidx=use_dst_gather)
    nc = _get_program(run, use_dst_gather, nq=4, egbufs=3,
                      split=2, lite16=True)
    in_maps = [dict(consts, **pc) for pc in per_core]
    res = run_bass_kernel_spmd(nc, in_maps, core_ids=list(range(CORES)))
    out = np.concatenate([r["out"].T for r in res.results], axis=0)[:N]
    return np.ascontiguousarray(out)
